# revision 14
# baseline (speedup 1.0000x reference)
"""Trainium2 Bass kernel for nn_AttentionBlock (b=1, c=1024, l=2048, 16 heads).

Sharding: 2 heads per core across 8 cores. Each core:
  - loads full x (bf16, 16-way-split DMA), computes GroupNorm stats paced by
    the DMA, folds the affine into bf16 qkv weights,
  - computes its 2 heads' q/k/v (bf16 outputs),
  - runs fused attention transposed (attT[s,t]) in two query halves with both
    heads' K=64 QK matmuls issued back-to-back at PE row strips (0,0)/(64,0)
    so they run concurrently.  The relative-position bias saturates for
    |s-t| >= 50, so only a 256-wide diagonal strip per s-tile gets an exact
    bias deposit (small table, identity matmul); outside the strip the
    constant bias is folded as exp(C) into pre-scaled AV weight variants
    (ones column included, so softmax denominators stay exact),
  - exp runs mostly on ACT, with a configurable fraction on DVE via a
    Schraudolph int16/bf16 bit-trick,
  - AV stacks both heads into [128]-col weights (zero-padded) so head1's
    output lands on partitions 64..127 directly; denominators ride cols 64/63,
  - output projection is a single K=128 matmul per tile on the stacked
    normalized heads; partials stream out per half, overlapped with the other
    half's attention.
Host sums the 8 partials and adds b_proj and the residual x.
"""

import math
import os
import numpy as np

N_HEAD = 16
NUM_BUCKETS = 32
MAX_DISTANCE = 64
GN_GROUPS = 32
GN_EPS = 1e-5

B, C, L = 1, 1024, 2048
DH = C // N_HEAD              # 64
N_CORES = 8
LT = L // 128                 # 16 l-tiles
CT = C // 128                 # 8 channel tiles
TBW2 = 256                    # strip bias table width
SCALE = 1.0 / math.sqrt(math.sqrt(DH))

# Schraudolph exp -> bf16 bits via int16: round(a*x + b), a = 128/ln2
SCHR_A = 128.0 / math.log(2.0)
SCHR_B = 16256.0 - 5.51

KDVE8 = int(os.environ.get("KDVE8", "3"))     # n of 8 exp tiles on DVE

_CACHE = {}


def _bucket_np(rel):
    # faithful numpy port of the reference _relative_position_bucket
    n = -rel
    nb = NUM_BUCKETS // 2
    ret = (n < 0).astype(np.int32) * nb
    n = np.abs(n)
    max_exact = nb // 2
    is_small = n < max_exact
    val_if_large = max_exact + (
        np.log(np.maximum(n, 1).astype(np.float32) / max_exact)
        / np.float32(math.log(MAX_DISTANCE / max_exact))
        * (nb - max_exact)
    ).astype(np.int32)
    val_if_large = np.minimum(val_if_large, nb - 1)
    return ret + np.where(is_small, n, val_if_large)


def _build_nc():
    import concourse.bacc as bacc
    import concourse.tile as tile
    from concourse import mybir

    F32 = mybir.dt.float32
    BF16 = mybir.dt.bfloat16
    I16 = mybir.dt.int16
    AF = mybir.ActivationFunctionType
    ALU = mybir.AluOpType

    nc = bacc.Bacc("TRN2", target_bir_lowering=False, debug=False,
                   num_devices=N_CORES)

    F8 = mybir.dt.float8e4
    d_x = nc.dram_tensor("x", [C, L], F8, kind="ExternalInput")
    d_wqkvT = nc.dram_tensor("wqkvT", [C, 384], BF16, kind="ExternalInput")
    d_consts = nc.dram_tensor("consts", [128, 27], F32, kind="ExternalInput")
    d_wproj2 = nc.dram_tensor("wproj2", [128, C], BF16, kind="ExternalInput")
    d_tb2 = nc.dram_tensor("tb2", [2, 128, TBW2], BF16, kind="ExternalInput")
    d_identb = nc.dram_tensor("identb", [128, 128], BF16, kind="ExternalInput")
    d_indT = nc.dram_tensor("indT", [4, 128], F32, kind="ExternalInput")
    d_out = nc.dram_tensor("pout", [C, L], BF16, kind="ExternalOutput")

    with tile.TileContext(nc) as tc:
        with tc.tile_pool(name="big", bufs=1) as big, \
             tc.tile_pool(name="small", bufs=1) as small:

            # ---- persistent SBUF tiles
            t_xb = big.tile([128, CT, L], F8)
            t_wqkvT = big.tile([128, CT, 384], BF16)
            t_wqkvS = big.tile([128, CT, 384], BF16)
            t_consts = small.tile([128, 27], F32)
            t_wproj2 = big.tile([128, C], BF16)
            t_tb2 = small.tile([128, 2, TBW2], BF16)
            t_identb = small.tile([128, 128], BF16)
            t_indT = small.tile([4, 128], F32)
            t_eps = small.tile([128, 1], F32)
            t_q2 = big.tile([128, L], BF16)
            t_k2z0 = big.tile([128, L], BF16)
            t_k2z1 = big.tile([128, L], BF16)
            t_v2 = big.tile([128, L], BF16)
            t_outh = big.tile([128, L], BF16)
            # AV weights: [s-local, i, col] ; head0 cols 0:64 v + 64 ones,
            # head1 col 32 ones + cols 64:128 v; rest zero.  p=plain (strip),
            # a = *exp(C_A) (t <= s-50 region), b = *exp(C_B) (t >= s+50).
            vt = {0: {}, 1: {}}
            for j in range(2):
                for v in ("p", "a", "b"):
                    vt[j][v] = big.tile([128, LT, 128], BF16,
                                        name=f"vt{j}{v}")

            # ---- DMAs: small consts first, then x split 16 ways, then
            # weights, bias table last (needed latest).
            nc.sync.dma_start(out=t_consts[:], in_=d_consts[:])
            nc.sync.dma_start(out=t_identb[:], in_=d_identb[:])
            nc.sync.dma_start(out=t_indT[:], in_=d_indT[:])
            xr = d_x[:].rearrange("(t p) l -> p t l", p=128)
            for t in range(CT):
                for hl in range(2):
                    nc.sync.dma_start(
                        out=t_xb[:, t, hl * 1024:(hl + 1) * 1024],
                        in_=xr[:, t, hl * 1024:(hl + 1) * 1024])
            nc.sync.dma_start(
                out=t_wqkvT[:],
                in_=d_wqkvT[:].rearrange("(t p) m -> p t m", p=128))
            nc.sync.dma_start(out=t_wproj2[:], in_=d_wproj2[:])
            nc.sync.dma_start(out=t_tb2[:],
                              in_=d_tb2[:].rearrange("j p m -> p j m"))
            t_ind = t_consts[:, 0:4]
            t_gnw = t_consts[:, 4:12]
            t_gnb = t_consts[:, 12:20]
            t_bvec = t_consts[:, 20:23]
            nc.vector.memset(t_eps[:], GN_EPS)
            # zero the plain AV weight tiles (variants inherit via scaling)
            nc.vector.memset(vt[0]["p"][:], 0.0)
            nc.vector.memset(vt[1]["p"][:], 0.0)
            nc.vector.memset(t_k2z0[64:128, :], 0.0)
            nc.vector.memset(t_k2z1[0:64, :], 0.0)
            nc.vector.memset(vt[0]["p"][:, :, 64:65], 1.0)
            nc.vector.memset(vt[1]["p"][:, :, 32:33], 1.0)
            t_ones = small.tile([1, 1024], BF16)
            t_zrow = small.tile([1, 128], BF16)
            t_sel = small.tile([33, 128], BF16)
            t_dnp = [small.tile([33, 512], BF16, name=f"t_dn{i}")
                     for i in range(2)]
            nc.vector.memset(t_ones[:], 1.0)
            nc.vector.memset(t_zrow[:], 0.0)
            nc.vector.memset(t_sel[:], 0.0)
            nc.vector.memset(t_sel[0:1, 0:64], 1.0)
            nc.vector.memset(t_sel[32:33, 64:128], 1.0)
            nc.vector.memset(t_dnp[0][:], 0.0)
            nc.vector.memset(t_dnp[1][:], 0.0)

            # ---- GroupNorm stats paced by x DMA + PE warmup dummies.
            with tc.tile_pool(name="warm_ps", bufs=1, space="PSUM") as warm_ps, \
                 tc.tile_pool(name="gn_ps", bufs=2, space="PSUM") as gn_ps, \
                 tc.tile_pool(name="gn_sb", bufs=2) as gn_sb:
                t_warm = warm_ps.tile([128, 512], F32)
                # table preload for Exp while DMA streams
                nc.scalar.activation(out=t_warm[0:1, 0:1], in_=t_eps[0:1, :],
                                     func=AF.Exp)
                sall = gn_sb.tile([128, CT], F32)
                sqall = gn_sb.tile([128, CT], F32)
                for t in range(CT):
                    scr = gn_sb.tile([128, L], BF16, tag="scr")
                    nc.vector.tensor_scalar(out=scr[:], in0=t_xb[:, t, :],
                                            scalar1=1.0, scalar2=0.0,
                                            op0=ALU.mult, op1=ALU.add,
                                            accum_out=sall[:, t:t + 1])
                    scra = gn_sb.tile([128, L], BF16, tag="scra")
                    nc.scalar.activation(out=scra[:], in_=t_xb[:, t, :],
                                         func=AF.Square,
                                         accum_out=sqall[:, t:t + 1])
                    # keep PE ticking through the DMA phase (HAM warmup)
                    for r in range(2):
                        nc.tensor.matmul(
                            out=t_warm[:], lhsT=t_identb[:],
                            rhs=t_xb[:, t, r * 512:(r + 1) * 512],
                            start=True, stop=True, skip_group_check=True)
                v2sall = gn_sb.tile([128, 2 * CT], F32)
                nc.vector.tensor_scalar_mul(out=v2sall[:, 0:CT], in0=sall[:],
                                            scalar1=1.0 / L)
                nc.vector.tensor_scalar_mul(out=v2sall[:, CT:], in0=sqall[:],
                                            scalar1=1.0 / L)
                p_g4 = gn_ps.tile([4, 2 * CT], F32)
                nc.tensor.matmul(out=p_g4[:], lhsT=t_ind, rhs=v2sall[:],
                                 start=True, stop=True)
                sc24 = gn_sb.tile([4, 2 * CT], F32)
                nc.vector.tensor_scalar_mul(out=sc24[:], in0=p_g4[:],
                                            scalar1=1.0 / 32.0)
                gs2 = gn_sb.tile([4, 2 * CT], F32)
                nc.vector.tensor_mul(out=gs2[:, 0:CT], in0=sc24[:, 0:CT],
                                     in1=sc24[:, 0:CT])
                nc.vector.tensor_sub(out=gs2[:, CT:], in0=sc24[:, CT:],
                                     in1=gs2[:, 0:CT])
                nc.scalar.activation(out=gs2[:, CT:], in_=gs2[:, CT:],
                                     func=AF.Sqrt, bias=t_eps[0:4, :])
                nc.vector.reciprocal(out=gs2[:, CT:], in_=gs2[:, CT:])
                nc.vector.tensor_copy(out=gs2[:, 0:CT], in_=sc24[:, 0:CT])
                p_c2 = gn_ps.tile([128, 2 * CT], F32)
                nc.tensor.matmul(out=p_c2[:], lhsT=t_indT[:], rhs=gs2[:],
                                 start=True, stop=True)
                svec = gn_sb.tile([128, CT], F32)
                nc.vector.tensor_mul(out=svec[:], in0=p_c2[:, CT:], in1=t_gnw)
                mub = small.tile([128, CT], BF16)
                nc.vector.tensor_copy(out=mub[:], in_=p_c2[:, 0:CT])
                for t in range(CT):
                    nc.vector.tensor_scalar_mul(
                        out=t_wqkvS[:, t, :], in0=t_wqkvT[:, t, :],
                        scalar1=svec[:, t:t + 1])

            # ---- qkv projection (chunk-major), v transposed per chunk.
            t_cb = small.tile([128, 3], F32)
            with tc.tile_pool(name="qkv_ps", bufs=4, space="PSUM") as qkv_ps, \
                 tc.tile_pool(name="vt_ps", bufs=2, space="PSUM") as vt_ps, \
                 tc.tile_pool(name="gn_ps2", bufs=1, space="PSUM") as gn_ps2:

                def emit_copy(p, ci, dst, nn):
                    sl = slice(nn * 512, (nn + 1) * 512)
                    if ci == 1:
                        nc.vector.tensor_scalar(
                            out=t_k2z0[0:64, sl], in0=p[0:64, :],
                            scalar1=t_cb[0:64, 1:2], scalar2=None, op0=ALU.add)
                        nc.vector.tensor_scalar(
                            out=t_k2z1[64:128, sl], in0=p[64:128, :],
                            scalar1=t_cb[64:128, 1:2], scalar2=None, op0=ALU.add)
                    else:
                        nc.vector.tensor_scalar(
                            out=dst[:, sl],
                            in0=p[:], scalar1=t_cb[:, ci:ci + 1], scalar2=None,
                            op0=ALU.add)

                pending = []
                for nn in range(4):
                    for ci, dst in ((0, t_q2), (1, None), (2, t_v2)):
                        p = qkv_ps.tile([128, 512], F32, tag="qkv")
                        for kt in range(CT):
                            nc.tensor.matmul(
                                out=p[:],
                                lhsT=t_wqkvS[:, kt, ci * 128:(ci + 1) * 128],
                                rhs=t_xb[:, kt, nn * 512:(nn + 1) * 512],
                                start=(kt == 0), stop=(kt == CT - 1))
                        if nn == 0:
                            pending.append((p, ci, dst, nn))
                        else:
                            emit_copy(p, ci, dst, nn)
                    if nn == 0:
                        # const[o] = sum_c W''[c, o] * mu_c
                        p_cn = gn_ps2.tile([128, 3], F32)
                        for cj in range(3):
                            for kt in range(CT):
                                nc.tensor.matmul(
                                    out=p_cn[:, cj:cj + 1],
                                    lhsT=t_wqkvS[:, kt, cj * 128:(cj + 1) * 128],
                                    rhs=mub[:, kt:kt + 1],
                                    start=(kt == 0), stop=(kt == CT - 1),
                                    skip_group_check=True)
                        nc.vector.tensor_sub(out=t_cb[:], in0=t_bvec, in1=p_cn[:])
                        for args in pending:
                            emit_copy(*args)
                        pending = []
                    # transpose the 4 finished v chunks into the AV weights
                    for sub in range(4):
                        i = nn * 4 + sub
                        pt = vt_ps.tile([128, 128], BF16, tag="vt")
                        nc.tensor.transpose(out=pt[:],
                                            in_=t_v2[:, i * 128:(i + 1) * 128],
                                            identity=t_identb[:])
                        nc.vector.tensor_copy(out=vt[0]["p"][:, i, 0:64],
                                              in_=pt[:, 0:64])
                        nc.vector.tensor_copy(out=vt[1]["p"][:, i, 64:128],
                                              in_=pt[:, 64:128])
                # scaled AV weight variants (fold exp(C) of the constant-bias
                # regions, ones column included)
                for j, v, col in ((0, "a", 23), (0, "b", 24),
                                  (1, "a", 25), (1, "b", 26)):
                    nc.vector.tensor_scalar_mul(
                        out=vt[j][v][:], in0=vt[j]["p"][:],
                        scalar1=t_consts[:, col:col + 1])

            # ---- attention in four query quarters; AV accumulators are
            # double-buffered [128,512] tiles so norm/proj of quarter q
            # overlaps quarter q+1's attention with no PE bubble.
            with tc.tile_pool(name="att_ps", bufs=4, space="PSUM") as att_ps, \
                 tc.tile_pool(name="av_ps", bufs=2, space="PSUM") as av_ps, \
                 tc.tile_pool(name="expp", bufs=6) as expp, \
                 tc.tile_pool(name="bcp", bufs=2) as bcp, \
                 tc.tile_pool(name="outp", bufs=3) as outp:

                cnt = [0]

                def emit_av(i, c0, es, p_avs):
                    lo = max(0, 128 * i - 64)
                    hi = min(L, 128 * i + 192)
                    rng = ((c0, min(c0 + 512, lo), "a"),
                           (max(c0, lo), min(c0 + 512, hi), "p"),
                           (max(c0, hi), c0 + 512, "b"))
                    for j in (0, 1):
                        for (r0, r1, v) in rng:
                            if r1 <= r0:
                                continue
                            nc.tensor.matmul(
                                out=p_avs[j][:, r0 - c0:r1 - c0],
                                lhsT=vt[j][v][:, i, :],
                                rhs=es[j][:, r0 - c0:r1 - c0],
                                start=False, stop=(i == LT - 1),
                                skip_group_check=True)

                def att_quarter(q, projq, norm_thunk=None):
                    c0 = 512 * q
                    p_av0 = av_ps.tile([128, 512], F32, tag="av0")
                    p_av1 = av_ps.tile([128, 512], F32, tag="av1")
                    p_avs = (p_av0, p_av1)
                    # zero-init accumulators (uniform has_written state)
                    for pa in p_avs:
                        nc.tensor.matmul(
                            out=pa[:], lhsT=t_zrow[0:1, :],
                            rhs=t_ones[0:1, 0:512],
                            start=True, stop=False, skip_group_check=True)
                    pend = []
                    for i in range(LT):
                        lo = max(0, 128 * i - 64)
                        hi = min(L, 128 * i + 192)
                        a = max(c0, lo)
                        b = min(c0 + 512, hi)
                        dep = b > a
                        atts = []
                        for j, kz in ((0, t_k2z0), (1, t_k2z1)):
                            at = att_ps.tile([128, 512], F32, tag="att")
                            nc.tensor.matmul(
                                out=at[:],
                                lhsT=kz[:, 128 * i:128 * (i + 1)],
                                rhs=t_q2[:, c0:c0 + 512],
                                start=True, stop=not dep,
                                skip_group_check=True)
                            atts.append(at)
                        if dep:
                            for j in (0, 1):
                                nc.tensor.matmul(
                                    out=atts[j][:, a - c0:b - c0],
                                    lhsT=t_identb[:],
                                    rhs=t_tb2[:, j, a - (128 * i - 64):
                                              b - (128 * i - 64)],
                                    start=False, stop=True,
                                    skip_group_check=True)
                        es = []
                        for j in (0, 1):
                            e = expp.tile([128, 512], BF16, tag="exp")
                            if (cnt[0] * KDVE8) % 8 < KDVE8:
                                nc.vector.tensor_scalar(
                                    out=e[:].bitcast(I16), in0=atts[j][:],
                                    scalar1=SCHR_A, scalar2=SCHR_B,
                                    op0=ALU.mult, op1=ALU.add)
                            else:
                                nc.scalar.activation(out=e[:], in_=atts[j][:],
                                                     func=AF.Exp)
                            cnt[0] += 1
                            es.append(e)
                        pend.append((i, c0, es))
                        if norm_thunk and i == 2:
                            norm_thunk()
                            norm_thunk = None
                        if len(pend) > 2:
                            emit_av(*pend.pop(0), p_avs)
                        if projq and 4 <= i < 12:
                            projq.pop(0)()
                    for args in pend:
                        emit_av(*args, p_avs)
                    while projq:
                        projq.pop(0)()
                    return p_avs

                def emit_norm(q, p_avs):
                    c0 = 512 * q
                    t_dn = t_dnp[q % 2]
                    t_bc = bcp.tile([128, 512], F32, tag="bc")
                    nc.vector.tensor_copy(out=t_dn[0:1, :],
                                          in_=p_avs[0][64:65, :])
                    nc.scalar.copy(out=t_dn[32:33, :],
                                   in_=p_avs[1][32:33, :])
                    pb = att_ps.tile([128, 512], F32, tag="att")
                    nc.tensor.matmul(out=pb[:], lhsT=t_sel[:], rhs=t_dn[:],
                                     start=True, stop=True,
                                     skip_group_check=True)
                    nc.vector.reciprocal(out=t_bc[:], in_=pb[:])
                    nc.vector.tensor_mul(out=t_outh[0:64, c0:c0 + 512],
                                         in0=p_avs[0][0:64, :],
                                         in1=t_bc[0:64, :])
                    nc.vector.tensor_mul(out=t_outh[64:128, c0:c0 + 512],
                                         in0=p_avs[1][64:128, :],
                                         in1=t_bc[64:128, :])

                def proj_thunks(q):
                    thunks = []
                    k = [0]
                    t0 = 512 * q
                    for mo in range(8):
                        def th(mo=mo):
                            p = att_ps.tile([128, 512], F32, tag="att")
                            nc.tensor.matmul(
                                out=p[:],
                                lhsT=t_wproj2[:, mo * 128:(mo + 1) * 128],
                                rhs=t_outh[:, t0:t0 + 512],
                                start=True, stop=True,
                                skip_group_check=True)
                            po = outp.tile([128, 512], BF16, tag="po")
                            if k[0] % 2 == 1:
                                nc.scalar.copy(out=po[:], in_=p[:])
                            else:
                                nc.vector.tensor_copy(out=po[:], in_=p[:])
                            k[0] += 1
                            nc.sync.dma_start(
                                out=d_out[mo * 128:(mo + 1) * 128,
                                          t0:t0 + 512],
                                in_=po[:])
                        thunks.append(th)
                    return thunks

                prev = None
                for q in range(4):
                    nt = None
                    if prev is not None:
                        nt = (lambda qq=q - 1, a=prev: emit_norm(qq, a))
                    pj = proj_thunks(q - 1) if q > 0 else None
                    prev = att_quarter(q, pj, nt)
                emit_norm(3, prev)
                for th in proj_thunks(3):
                    th()

    nc.compile()
    return nc


def _host_inputs(x, gn_w, gn_b, w_qkv, b_qkv, w_proj, b_proj, rel_bias):
    import ml_dtypes
    x2 = np.ascontiguousarray(x.reshape(C, L)).astype(np.float32)
    identb = np.eye(128).astype(ml_dtypes.bfloat16)
    ind = np.zeros((128, 4), dtype=np.float32)
    for p in range(128):
        ind[p, p // 32] = 1.0
    indT = np.ascontiguousarray(ind.T)
    gnw = np.ascontiguousarray(np.asarray(gn_w, np.float32).reshape(CT, 128).T)
    gnb = np.ascontiguousarray(np.asarray(gn_b, np.float32).reshape(CT, 128).T)

    w_qkv = np.asarray(w_qkv, np.float32)
    b_qkv = np.asarray(b_qkv, np.float32)
    w_proj = np.asarray(w_proj, np.float32)
    rel_bias = np.asarray(rel_bias, np.float32)

    # Toeplitz diag values D_h[u] = 8 * rel_bias[bucket(u - (L-1)), h]
    u = np.arange(2 * L - 1, dtype=np.int64)
    buckets = _bucket_np((u - (L - 1)).astype(np.int32))
    # strip table: tb2[p, m'] = D[p - m' + 2111]
    p_idx = np.arange(128)[:, None]
    m_idx = np.arange(TBW2)[None, :]
    tb2_arg = p_idx - m_idx + (64 + L - 1)

    in_maps = []
    for d in range(N_CORES):
        heads = (2 * d, 2 * d + 1)
        wq, wk, wv, bq, bk, bv = [], [], [], [], [], []
        for h in heads:
            base = h * 3 * DH
            wq.append(w_qkv[base:base + DH] * SCALE)
            wk.append(w_qkv[base + DH:base + 2 * DH] * SCALE)
            wv.append(w_qkv[base + 2 * DH:base + 3 * DH])
            bq.append(b_qkv[base:base + DH] * SCALE)
            bk.append(b_qkv[base + DH:base + 2 * DH] * SCALE)
            bv.append(b_qkv[base + 2 * DH:base + 3 * DH])
        wall = np.concatenate(wq + wk + wv, axis=0)        # [384, 1024]
        wqkvT = np.ascontiguousarray(wall.T)               # [1024, 384]
        bvec = np.stack([np.concatenate(bq), np.concatenate(bk),
                         np.concatenate(bv)], axis=1)       # [128, 3]
        gnb_contrib = wall @ np.asarray(gn_b, np.float32)   # [384]
        bvec = bvec + gnb_contrib.reshape(3, 128).T
        # stacked proj weights: rows = [head0 dims, head1 dims]
        wproj2 = np.concatenate(
            [np.ascontiguousarray(w_proj[:, h * DH:(h + 1) * DH].T)
             for h in heads], axis=0)                       # [128, 1024]
        # per-head saturated-bias exp factors: C_A for t<=s-50 (bucket 31),
        # C_B for t>=s+50 (bucket 15)
        vs = []
        for h in heads:
            vs.append(math.exp(8.0 * rel_bias[31, h]))
            vs.append(math.exp(8.0 * rel_bias[15, h]))
        vscale = np.tile(np.array(vs, np.float32)[None, :], (128, 1))
        tb2 = np.stack(
            [(8.0 * rel_bias[buckets, h])[tb2_arg] for h in heads],
            axis=0).astype(ml_dtypes.bfloat16)              # [2, 128, 256]
        consts = np.concatenate([ind, gnw, gnb, bvec.astype(np.float32),
                                 vscale], axis=1).astype(np.float32)
        in_maps.append({
            "x": x2.astype(ml_dtypes.float8_e4m3fn),
            "wqkvT": wqkvT.astype(ml_dtypes.bfloat16),
            "consts": consts,
            "wproj2": wproj2.astype(ml_dtypes.bfloat16),
            "tb2": tb2, "identb": identb, "indT": indT,
        })
    return in_maps


def kernel(x, gn_w, gn_b, w_qkv, b_qkv, w_proj, b_proj, rel_bias, **run_kwargs):
    from concourse.bass_utils import run_bass_kernel_spmd
    if "nc" not in _CACHE:
        _CACHE["nc"] = _build_nc()
    nc = _CACHE["nc"]
    in_maps = _host_inputs(x, gn_w, gn_b, w_qkv, b_qkv, w_proj, b_proj, rel_bias)
    res = run_bass_kernel_spmd(nc, in_maps, core_ids=list(range(N_CORES)),
                               **run_kwargs)
    _CACHE["last_result"] = res
    acc = np.zeros((C, L), dtype=np.float32)
    for d in range(N_CORES):
        acc += np.asarray(res.results[d]["pout"], dtype=np.float32)
    out = acc + np.asarray(b_proj, np.float32)[:, None] \
        + np.asarray(x, np.float32).reshape(C, L)
    return out.reshape(B, C, L)


# revision 16
# speedup vs baseline: 1.0336x; 1.0336x over previous
"""Trainium2 Bass kernel for nn_AttentionBlock (b=1, c=1024, l=2048, 16 heads).

Sharding: 2 heads per core across 8 cores. Each core:
  - loads full x (bf16, 16-way-split DMA), computes GroupNorm stats paced by
    the DMA, folds the affine into bf16 qkv weights,
  - computes its 2 heads' q/k/v (bf16 outputs),
  - runs fused attention transposed (attT[s,t]) in two query halves with both
    heads' K=64 QK matmuls issued back-to-back at PE row strips (0,0)/(64,0)
    so they run concurrently.  The relative-position bias saturates for
    |s-t| >= 50, so only a 256-wide diagonal strip per s-tile gets an exact
    bias deposit (small table, identity matmul); outside the strip the
    constant bias is folded as exp(C) into pre-scaled AV weight variants
    (ones column included, so softmax denominators stay exact),
  - exp runs mostly on ACT, with a configurable fraction on DVE via a
    Schraudolph int16/bf16 bit-trick,
  - AV stacks both heads into [128]-col weights (zero-padded) so head1's
    output lands on partitions 64..127 directly; denominators ride cols 64/63,
  - output projection is a single K=128 matmul per tile on the stacked
    normalized heads; partials stream out per half, overlapped with the other
    half's attention.
Host sums the 8 partials and adds b_proj and the residual x.
"""

import math
import os
import numpy as np

N_HEAD = 16
NUM_BUCKETS = 32
MAX_DISTANCE = 64
GN_GROUPS = 32
GN_EPS = 1e-5

B, C, L = 1, 1024, 2048
DH = C // N_HEAD              # 64
N_CORES = 8
LT = L // 128                 # 16 l-tiles
CT = C // 128                 # 8 channel tiles
TBW2 = 256                    # strip bias table width
SCALE = 1.0 / math.sqrt(math.sqrt(DH))

# Schraudolph exp -> bf16 bits via int16: round(a*x + b), a = 128/ln2
SCHR_A = 128.0 / math.log(2.0)
SCHR_B = 16256.0 - 5.51

KDVE8 = int(os.environ.get("KDVE8", "3"))     # n of 8 exp tiles on DVE

_CACHE = {}


def _bucket_np(rel):
    # faithful numpy port of the reference _relative_position_bucket
    n = -rel
    nb = NUM_BUCKETS // 2
    ret = (n < 0).astype(np.int32) * nb
    n = np.abs(n)
    max_exact = nb // 2
    is_small = n < max_exact
    val_if_large = max_exact + (
        np.log(np.maximum(n, 1).astype(np.float32) / max_exact)
        / np.float32(math.log(MAX_DISTANCE / max_exact))
        * (nb - max_exact)
    ).astype(np.int32)
    val_if_large = np.minimum(val_if_large, nb - 1)
    return ret + np.where(is_small, n, val_if_large)


def _build_nc():
    import concourse.bacc as bacc
    import concourse.tile as tile
    from concourse import mybir

    F32 = mybir.dt.float32
    BF16 = mybir.dt.bfloat16
    I16 = mybir.dt.int16
    AF = mybir.ActivationFunctionType
    ALU = mybir.AluOpType

    nc = bacc.Bacc("TRN2", target_bir_lowering=False, debug=False,
                   num_devices=N_CORES)

    F8 = mybir.dt.float8e4
    d_x = nc.dram_tensor("x", [C, L], F8, kind="ExternalInput")
    d_wqkvT = nc.dram_tensor("wqkvT", [C, 384], BF16, kind="ExternalInput")
    d_consts = nc.dram_tensor("consts", [128, 27], F32, kind="ExternalInput")
    d_wproj2 = nc.dram_tensor("wproj2", [128, C], BF16, kind="ExternalInput")
    d_tb2 = nc.dram_tensor("tb2", [2, 128, TBW2], BF16, kind="ExternalInput")
    d_identb = nc.dram_tensor("identb", [128, 128], BF16, kind="ExternalInput")
    d_indT = nc.dram_tensor("indT", [4, 128], F32, kind="ExternalInput")
    d_out = nc.dram_tensor("pout", [C, L], BF16, kind="ExternalOutput")

    with tile.TileContext(nc) as tc:
        with tc.tile_pool(name="big", bufs=1) as big, \
             tc.tile_pool(name="small", bufs=1) as small:

            # ---- persistent SBUF tiles
            t_xb = big.tile([128, CT, L], F8)
            t_wqkvT = big.tile([128, CT, 384], BF16)
            t_wqkvS = big.tile([128, CT, 384], BF16)
            t_consts = small.tile([128, 27], F32)
            t_wproj2 = big.tile([128, C], BF16)
            t_tb2 = small.tile([128, 2, TBW2], BF16)
            t_identb = small.tile([128, 128], BF16)
            t_indT = small.tile([4, 128], F32)
            t_eps = small.tile([128, 1], F32)
            t_q2 = big.tile([128, L], BF16)
            t_k2z0 = big.tile([128, L], BF16)
            t_k2z1 = big.tile([128, L], BF16)
            t_v2 = big.tile([128, L], BF16)
            t_outh = big.tile([128, L], BF16)
            # AV weights: [s-local, i, col] ; head0 cols 0:64 v + 64 ones,
            # head1 col 32 ones + cols 64:128 v; rest zero.  p=plain (strip),
            # a = *exp(C_A) (t <= s-50 region), b = *exp(C_B) (t >= s+50).
            vt = {0: {}, 1: {}}
            for j in range(2):
                for v in ("p", "a", "b"):
                    vt[j][v] = big.tile([128, LT, 128], BF16,
                                        name=f"vt{j}{v}")

            # ---- DMAs: small consts first, then x split 16 ways, then
            # weights, bias table last (needed latest).
            nc.sync.dma_start(out=t_consts[:], in_=d_consts[:])
            nc.sync.dma_start(out=t_identb[:], in_=d_identb[:])
            nc.sync.dma_start(out=t_indT[:], in_=d_indT[:])
            xr = d_x[:].rearrange("(t p) l -> p t l", p=128)
            for t in range(CT):
                for ph in range(2):
                    psl = slice(64 * ph, 64 * (ph + 1))
                    nc.sync.dma_start(out=t_xb[psl, t, :],
                                      in_=xr[psl, t, :])
            nc.sync.dma_start(
                out=t_wqkvT[:],
                in_=d_wqkvT[:].rearrange("(t p) m -> p t m", p=128))
            nc.sync.dma_start(out=t_wproj2[:], in_=d_wproj2[:])
            nc.sync.dma_start(out=t_tb2[:],
                              in_=d_tb2[:].rearrange("j p m -> p j m"))
            t_ind = t_consts[:, 0:4]
            t_gnw = t_consts[:, 4:12]
            t_gnb = t_consts[:, 12:20]
            t_bvec = t_consts[:, 20:23]
            nc.vector.memset(t_eps[:], GN_EPS)
            # zero the plain AV weight tiles (variants inherit via scaling);
            # on gpsimd so the DVE queue stays free for GroupNorm stats
            nc.gpsimd.memset(vt[0]["p"][:], 0.0)
            nc.gpsimd.memset(vt[1]["p"][:], 0.0)
            nc.gpsimd.memset(t_k2z0[64:128, :], 0.0)
            nc.gpsimd.memset(t_k2z1[0:64, :], 0.0)
            nc.gpsimd.memset(vt[0]["p"][:, :, 64:65], 1.0)
            nc.gpsimd.memset(vt[1]["p"][:, :, 32:33], 1.0)
            t_ones = small.tile([1, 1024], BF16)
            t_zrow = small.tile([1, 128], BF16)
            t_sel = small.tile([33, 128], BF16)
            t_dnp = [small.tile([33, 512], BF16, name=f"t_dn{i}")
                     for i in range(2)]
            nc.vector.memset(t_ones[:], 1.0)
            nc.vector.memset(t_zrow[:], 0.0)
            nc.gpsimd.memset(t_sel[:], 0.0)
            nc.gpsimd.memset(t_sel[0:1, 0:64], 1.0)
            nc.gpsimd.memset(t_sel[32:33, 64:128], 1.0)
            nc.gpsimd.memset(t_dnp[0][:], 0.0)
            nc.gpsimd.memset(t_dnp[1][:], 0.0)

            # ---- GroupNorm stats paced by x DMA + PE warmup dummies.
            with tc.tile_pool(name="warm_ps", bufs=1, space="PSUM") as warm_ps, \
                 tc.tile_pool(name="gn_ps", bufs=2, space="PSUM") as gn_ps, \
                 tc.tile_pool(name="gn_sb", bufs=2) as gn_sb:
                t_warm = warm_ps.tile([128, 512], F32)
                # table preload for Exp while DMA streams
                nc.scalar.activation(out=t_warm[0:1, 0:1], in_=t_eps[0:1, :],
                                     func=AF.Exp)
                sall = gn_sb.tile([128, CT], F32)
                sqall = gn_sb.tile([128, CT], F32)
                for t in range(CT):
                    scr = gn_sb.tile([128, L], BF16, tag="scr")
                    nc.vector.tensor_scalar(out=scr[:], in0=t_xb[:, t, :],
                                            scalar1=1.0, scalar2=0.0,
                                            op0=ALU.mult, op1=ALU.add,
                                            accum_out=sall[:, t:t + 1])
                    scra = gn_sb.tile([128, L], BF16, tag="scra")
                    nc.scalar.activation(out=scra[:], in_=t_xb[:, t, :],
                                         func=AF.Square,
                                         accum_out=sqall[:, t:t + 1])
                    # keep PE ticking through the DMA phase (HAM warmup)
                    for r in range(2):
                        nc.tensor.matmul(
                            out=t_warm[:], lhsT=t_identb[:],
                            rhs=t_xb[:, t, r * 512:(r + 1) * 512],
                            start=True, stop=True, skip_group_check=True)
                v2sall = gn_sb.tile([128, 2 * CT], F32)
                nc.vector.tensor_scalar_mul(out=v2sall[:, 0:CT], in0=sall[:],
                                            scalar1=1.0 / L)
                nc.vector.tensor_scalar_mul(out=v2sall[:, CT:], in0=sqall[:],
                                            scalar1=1.0 / L)
                p_g4 = gn_ps.tile([4, 2 * CT], F32)
                nc.tensor.matmul(out=p_g4[:], lhsT=t_ind, rhs=v2sall[:],
                                 start=True, stop=True)
                sc24 = gn_sb.tile([4, 2 * CT], F32)
                nc.vector.tensor_scalar_mul(out=sc24[:], in0=p_g4[:],
                                            scalar1=1.0 / 32.0)
                gs2 = gn_sb.tile([4, 2 * CT], F32)
                nc.vector.tensor_mul(out=gs2[:, 0:CT], in0=sc24[:, 0:CT],
                                     in1=sc24[:, 0:CT])
                nc.vector.tensor_sub(out=gs2[:, CT:], in0=sc24[:, CT:],
                                     in1=gs2[:, 0:CT])
                nc.scalar.activation(out=gs2[:, CT:], in_=gs2[:, CT:],
                                     func=AF.Sqrt, bias=t_eps[0:4, :])
                nc.vector.reciprocal(out=gs2[:, CT:], in_=gs2[:, CT:])
                nc.vector.tensor_copy(out=gs2[:, 0:CT], in_=sc24[:, 0:CT])
                p_c2 = gn_ps.tile([128, 2 * CT], F32)
                nc.tensor.matmul(out=p_c2[:], lhsT=t_indT[:], rhs=gs2[:],
                                 start=True, stop=True)
                svec = gn_sb.tile([128, CT], F32)
                nc.vector.tensor_mul(out=svec[:], in0=p_c2[:, CT:], in1=t_gnw)
                mub = small.tile([128, CT], BF16)
                nc.vector.tensor_copy(out=mub[:], in_=p_c2[:, 0:CT])
                for t in range(CT):
                    nc.vector.tensor_scalar_mul(
                        out=t_wqkvS[:, t, :], in0=t_wqkvT[:, t, :],
                        scalar1=svec[:, t:t + 1])

            # ---- qkv projection (chunk-major), v transposed per chunk.
            t_cb = small.tile([128, 3], F32)
            with tc.tile_pool(name="qkv_ps", bufs=4, space="PSUM") as qkv_ps, \
                 tc.tile_pool(name="vt_ps", bufs=2, space="PSUM") as vt_ps, \
                 tc.tile_pool(name="gn_ps2", bufs=1, space="PSUM") as gn_ps2:

                def emit_copy(p, ci, dst, nn):
                    sl = slice(nn * 512, (nn + 1) * 512)
                    if ci == 1:
                        nc.vector.tensor_scalar(
                            out=t_k2z0[0:64, sl], in0=p[0:64, :],
                            scalar1=t_cb[0:64, 1:2], scalar2=None, op0=ALU.add)
                        nc.vector.tensor_scalar(
                            out=t_k2z1[64:128, sl], in0=p[64:128, :],
                            scalar1=t_cb[64:128, 1:2], scalar2=None, op0=ALU.add)
                    else:
                        nc.vector.tensor_scalar(
                            out=dst[:, sl],
                            in0=p[:], scalar1=t_cb[:, ci:ci + 1], scalar2=None,
                            op0=ALU.add)

                pending = []
                for nn in range(4):
                    for ci, dst in ((0, t_q2), (1, None), (2, t_v2)):
                        p = qkv_ps.tile([128, 512], F32, tag="qkv")
                        for kt in range(CT):
                            nc.tensor.matmul(
                                out=p[:],
                                lhsT=t_wqkvS[:, kt, ci * 128:(ci + 1) * 128],
                                rhs=t_xb[:, kt, nn * 512:(nn + 1) * 512],
                                start=(kt == 0), stop=(kt == CT - 1))
                        if nn == 0:
                            pending.append((p, ci, dst, nn))
                        else:
                            emit_copy(p, ci, dst, nn)
                    if nn == 0:
                        # const[o] = sum_c W''[c, o] * mu_c
                        p_cn = gn_ps2.tile([128, 3], F32)
                        for cj in range(3):
                            for kt in range(CT):
                                nc.tensor.matmul(
                                    out=p_cn[:, cj:cj + 1],
                                    lhsT=t_wqkvS[:, kt, cj * 128:(cj + 1) * 128],
                                    rhs=mub[:, kt:kt + 1],
                                    start=(kt == 0), stop=(kt == CT - 1),
                                    skip_group_check=True)
                        nc.vector.tensor_sub(out=t_cb[:], in0=t_bvec, in1=p_cn[:])
                        for args in pending:
                            emit_copy(*args)
                        pending = []
                    # transpose the 4 finished v chunks into the AV weights
                    for sub in range(4):
                        i = nn * 4 + sub
                        pt = vt_ps.tile([128, 128], BF16, tag="vt")
                        nc.tensor.transpose(out=pt[:],
                                            in_=t_v2[:, i * 128:(i + 1) * 128],
                                            identity=t_identb[:])
                        nc.vector.tensor_copy(out=vt[0]["p"][:, i, 0:64],
                                              in_=pt[:, 0:64])
                        nc.vector.tensor_copy(out=vt[1]["p"][:, i, 64:128],
                                              in_=pt[:, 64:128])
                # scaled AV weight variants (fold exp(C) of the constant-bias
                # regions, ones column included)
                for j, v, col in ((0, "a", 23), (0, "b", 24),
                                  (1, "a", 25), (1, "b", 26)):
                    nc.vector.tensor_scalar_mul(
                        out=vt[j][v][:], in0=vt[j]["p"][:],
                        scalar1=t_consts[:, col:col + 1])

            # ---- attention in four query quarters; AV accumulators are
            # double-buffered [128,512] tiles so norm/proj of quarter q
            # overlaps quarter q+1's attention with no PE bubble.
            with tc.tile_pool(name="att_ps", bufs=4, space="PSUM") as att_ps, \
                 tc.tile_pool(name="av_ps", bufs=2, space="PSUM") as av_ps, \
                 tc.tile_pool(name="expp", bufs=6) as expp, \
                 tc.tile_pool(name="bcp", bufs=2) as bcp, \
                 tc.tile_pool(name="outp", bufs=3) as outp:

                cnt = [0]

                def emit_av(i, c0, es, p_avs):
                    lo = max(0, 128 * i - 64)
                    hi = min(L, 128 * i + 192)
                    rng = ((c0, min(c0 + 512, lo), "a"),
                           (max(c0, lo), min(c0 + 512, hi), "p"),
                           (max(c0, hi), c0 + 512, "b"))
                    for j in (0, 1):
                        for (r0, r1, v) in rng:
                            if r1 <= r0:
                                continue
                            nc.tensor.matmul(
                                out=p_avs[j][:, r0 - c0:r1 - c0],
                                lhsT=vt[j][v][:, i, :],
                                rhs=es[j][:, r0 - c0:r1 - c0],
                                start=False, stop=(i == LT - 1),
                                skip_group_check=True)

                def att_quarter(q, projq, norm_thunk=None):
                    c0 = 512 * q
                    p_av0 = av_ps.tile([128, 512], F32, tag="av0")
                    p_av1 = av_ps.tile([128, 512], F32, tag="av1")
                    p_avs = (p_av0, p_av1)
                    # zero-init accumulators (uniform has_written state)
                    for pa in p_avs:
                        nc.tensor.matmul(
                            out=pa[:], lhsT=t_zrow[0:1, :],
                            rhs=t_ones[0:1, 0:512],
                            start=True, stop=False, skip_group_check=True)
                    pend = []
                    for i in range(LT):
                        lo = max(0, 128 * i - 64)
                        hi = min(L, 128 * i + 192)
                        a = max(c0, lo)
                        b = min(c0 + 512, hi)
                        dep = b > a
                        atts = []
                        for j, kz in ((0, t_k2z0), (1, t_k2z1)):
                            at = att_ps.tile([128, 512], F32, tag="att")
                            nc.tensor.matmul(
                                out=at[:],
                                lhsT=kz[:, 128 * i:128 * (i + 1)],
                                rhs=t_q2[:, c0:c0 + 512],
                                start=True, stop=not dep,
                                skip_group_check=True)
                            atts.append(at)
                        if dep:
                            for j in (0, 1):
                                nc.tensor.matmul(
                                    out=atts[j][:, a - c0:b - c0],
                                    lhsT=t_identb[:],
                                    rhs=t_tb2[:, j, a - (128 * i - 64):
                                              b - (128 * i - 64)],
                                    start=False, stop=True,
                                    skip_group_check=True)
                        es = []
                        for j in (0, 1):
                            e = expp.tile([128, 512], BF16, tag="exp")
                            if (cnt[0] * KDVE8) % 8 < KDVE8:
                                nc.vector.tensor_scalar(
                                    out=e[:].bitcast(I16), in0=atts[j][:],
                                    scalar1=SCHR_A, scalar2=SCHR_B,
                                    op0=ALU.mult, op1=ALU.add)
                            else:
                                nc.scalar.activation(out=e[:], in_=atts[j][:],
                                                     func=AF.Exp)
                            cnt[0] += 1
                            es.append(e)
                        pend.append((i, c0, es))
                        if norm_thunk and i == 2:
                            norm_thunk()
                            norm_thunk = None
                        if len(pend) > 2:
                            emit_av(*pend.pop(0), p_avs)
                        if projq and 4 <= i < 12:
                            projq.pop(0)()
                    for args in pend:
                        emit_av(*args, p_avs)
                    while projq:
                        projq.pop(0)()
                    return p_avs

                def emit_norm(q, p_avs):
                    c0 = 512 * q
                    t_dn = t_dnp[q % 2]
                    t_bc = bcp.tile([128, 512], F32, tag="bc")
                    nc.vector.tensor_copy(out=t_dn[0:1, :],
                                          in_=p_avs[0][64:65, :])
                    nc.scalar.copy(out=t_dn[32:33, :],
                                   in_=p_avs[1][32:33, :])
                    pb = att_ps.tile([128, 512], F32, tag="att")
                    nc.tensor.matmul(out=pb[:], lhsT=t_sel[:], rhs=t_dn[:],
                                     start=True, stop=True,
                                     skip_group_check=True)
                    nc.vector.reciprocal(out=t_bc[:], in_=pb[:])
                    nc.vector.tensor_mul(out=t_outh[0:64, c0:c0 + 512],
                                         in0=p_avs[0][0:64, :],
                                         in1=t_bc[0:64, :])
                    nc.vector.tensor_mul(out=t_outh[64:128, c0:c0 + 512],
                                         in0=p_avs[1][64:128, :],
                                         in1=t_bc[64:128, :])

                def proj_thunks(q):
                    thunks = []
                    k = [0]
                    t0 = 512 * q
                    for mo in range(8):
                        def th(mo=mo):
                            p = att_ps.tile([128, 512], F32, tag="att")
                            nc.tensor.matmul(
                                out=p[:],
                                lhsT=t_wproj2[:, mo * 128:(mo + 1) * 128],
                                rhs=t_outh[:, t0:t0 + 512],
                                start=True, stop=True,
                                skip_group_check=True)
                            po = outp.tile([128, 512], BF16, tag="po")
                            if k[0] % 2 == 1:
                                nc.scalar.copy(out=po[:], in_=p[:])
                            else:
                                nc.vector.tensor_copy(out=po[:], in_=p[:])
                            k[0] += 1
                            nc.sync.dma_start(
                                out=d_out[mo * 128:(mo + 1) * 128,
                                          t0:t0 + 512],
                                in_=po[:])
                        thunks.append(th)
                    return thunks

                prev = None
                for q in range(4):
                    nt = None
                    if prev is not None:
                        nt = (lambda qq=q - 1, a=prev: emit_norm(qq, a))
                    pj = proj_thunks(q - 1) if q > 0 else None
                    prev = att_quarter(q, pj, nt)
                emit_norm(3, prev)
                for th in proj_thunks(3):
                    th()

    nc.compile()
    return nc


def _host_inputs(x, gn_w, gn_b, w_qkv, b_qkv, w_proj, b_proj, rel_bias):
    import ml_dtypes
    x2 = np.ascontiguousarray(x.reshape(C, L)).astype(np.float32)
    identb = np.eye(128).astype(ml_dtypes.bfloat16)
    ind = np.zeros((128, 4), dtype=np.float32)
    for p in range(128):
        ind[p, p // 32] = 1.0
    indT = np.ascontiguousarray(ind.T)
    gnw = np.ascontiguousarray(np.asarray(gn_w, np.float32).reshape(CT, 128).T)
    gnb = np.ascontiguousarray(np.asarray(gn_b, np.float32).reshape(CT, 128).T)

    w_qkv = np.asarray(w_qkv, np.float32)
    b_qkv = np.asarray(b_qkv, np.float32)
    w_proj = np.asarray(w_proj, np.float32)
    rel_bias = np.asarray(rel_bias, np.float32)

    # Toeplitz diag values D_h[u] = 8 * rel_bias[bucket(u - (L-1)), h]
    u = np.arange(2 * L - 1, dtype=np.int64)
    buckets = _bucket_np((u - (L - 1)).astype(np.int32))
    # strip table: tb2[p, m'] = D[p - m' + 2111]
    p_idx = np.arange(128)[:, None]
    m_idx = np.arange(TBW2)[None, :]
    tb2_arg = p_idx - m_idx + (64 + L - 1)

    in_maps = []
    for d in range(N_CORES):
        heads = (2 * d, 2 * d + 1)
        wq, wk, wv, bq, bk, bv = [], [], [], [], [], []
        for h in heads:
            base = h * 3 * DH
            wq.append(w_qkv[base:base + DH] * SCALE)
            wk.append(w_qkv[base + DH:base + 2 * DH] * SCALE)
            wv.append(w_qkv[base + 2 * DH:base + 3 * DH])
            bq.append(b_qkv[base:base + DH] * SCALE)
            bk.append(b_qkv[base + DH:base + 2 * DH] * SCALE)
            bv.append(b_qkv[base + 2 * DH:base + 3 * DH])
        wall = np.concatenate(wq + wk + wv, axis=0)        # [384, 1024]
        wqkvT = np.ascontiguousarray(wall.T)               # [1024, 384]
        bvec = np.stack([np.concatenate(bq), np.concatenate(bk),
                         np.concatenate(bv)], axis=1)       # [128, 3]
        gnb_contrib = wall @ np.asarray(gn_b, np.float32)   # [384]
        bvec = bvec + gnb_contrib.reshape(3, 128).T
        # stacked proj weights: rows = [head0 dims, head1 dims]
        wproj2 = np.concatenate(
            [np.ascontiguousarray(w_proj[:, h * DH:(h + 1) * DH].T)
             for h in heads], axis=0)                       # [128, 1024]
        # per-head saturated-bias exp factors: C_A for t<=s-50 (bucket 31),
        # C_B for t>=s+50 (bucket 15)
        vs = []
        for h in heads:
            vs.append(math.exp(8.0 * rel_bias[31, h]))
            vs.append(math.exp(8.0 * rel_bias[15, h]))
        vscale = np.tile(np.array(vs, np.float32)[None, :], (128, 1))
        tb2 = np.stack(
            [(8.0 * rel_bias[buckets, h])[tb2_arg] for h in heads],
            axis=0).astype(ml_dtypes.bfloat16)              # [2, 128, 256]
        consts = np.concatenate([ind, gnw, gnb, bvec.astype(np.float32),
                                 vscale], axis=1).astype(np.float32)
        in_maps.append({
            "x": x2.astype(ml_dtypes.float8_e4m3fn),
            "wqkvT": wqkvT.astype(ml_dtypes.bfloat16),
            "consts": consts,
            "wproj2": wproj2.astype(ml_dtypes.bfloat16),
            "tb2": tb2, "identb": identb, "indT": indT,
        })
    return in_maps


def kernel(x, gn_w, gn_b, w_qkv, b_qkv, w_proj, b_proj, rel_bias, **run_kwargs):
    from concourse.bass_utils import run_bass_kernel_spmd
    if "nc" not in _CACHE:
        _CACHE["nc"] = _build_nc()
    nc = _CACHE["nc"]
    in_maps = _host_inputs(x, gn_w, gn_b, w_qkv, b_qkv, w_proj, b_proj, rel_bias)
    res = run_bass_kernel_spmd(nc, in_maps, core_ids=list(range(N_CORES)),
                               **run_kwargs)
    _CACHE["last_result"] = res
    acc = np.zeros((C, L), dtype=np.float32)
    for d in range(N_CORES):
        acc += np.asarray(res.results[d]["pout"], dtype=np.float32)
    out = acc + np.asarray(b_proj, np.float32)[:, None] \
        + np.asarray(x, np.float32).reshape(C, L)
    return out.reshape(B, C, L)
